# revision 1
# baseline (speedup 1.0000x reference)
"""Trainium2 Bass kernel for nn_DotAlphaModule (sparse attention alpha).

Strategy (8 NeuronCores, SPMD):
  - Shard nodes N=8192 -> 1024/core; edges processed k-major (e = k*1024+n).
  - Full raw node table [8192, 9*128] bf16 replicated to every core's DRAM;
    neighbor rows fetched on-device with gpsimd.dma_gather (token-major).
  - sh(u) computed on device token-major; per-edge sh factors applied via
    diagonal matrices D_m = diag(sh_m) (built by one tensor_tensor against a
    masked-identity constant) feeding PE "transpose-matmuls" G_m^T @ D_m that
    accumulate the combined features S_l feature-major in PSUM.
  - Radial MLP, LayerNorms, fc and alpha-dot all run feature-major; LN stats
    via ones-matmuls on PE, rsqrt via DVE fast reciprocal + ACT sqrt.
  - All heavy matmuls in bf16 with f32 PSUM accumulation.
"""
import os
import sys
from contextlib import ExitStack

sys.path.insert(0, "/opt/trn_rl_repo")

import numpy as np
import ml_dtypes

import concourse.bass as bass
import concourse.tile as tile
import concourse.mybir as mybir
from concourse import bacc
from concourse.bass_utils import run_bass_kernel_spmd

BF16 = ml_dtypes.bfloat16

N, K = 8192, 32
NCORES = 8
NN = N // NCORES           # 1024 nodes per core
E = NN * K                 # 32768 edges per core
NCH = 9 * 128              # 1152 table row elements
NH, HD = 8, 32             # heads, head dim
CHUNK = 512                # edges per inner chunk
NCHUNK = E // CHUNK        # 64
EPS = 1e-5

C0 = 0.28209479177387814
C1 = 0.4886025119029199
C2 = 0.6307831305050401
S3 = 1.7320508075688772
NEG = 0.2

F32 = mybir.dt.float32
BF = mybir.dt.bfloat16
I16 = mybir.dt.int16
AF = mybir.ActivationFunctionType

# Native Silu runs on HW but is unimplemented in CoreSim; the fallback uses
# Sigmoid + an explicit multiply (identical math).
SILU_NATIVE = False


def _bap(ap, newap):
    return bass.AP(tensor=ap.tensor, offset=ap.offset, ap=newap)


def _build_nc(kmax=K):
    nc = bacc.Bacc("TRN2")
    # inputs
    tbl = nc.declare_dram_parameter("tbl", [N, NCH], BF, isOutput=False)
    selftbl = nc.declare_dram_parameter("selftbl", [NN, NCH], BF, isOutput=False)
    idxw = nc.declare_dram_parameter("idxw", [128, K * (NN // 16)], I16, isOutput=False)
    xT = nc.declare_dram_parameter("xT", [128, E], BF, isOutput=False)
    evp = nc.declare_dram_parameter("evp", [128, (E // 128) * 3], F32, isOutput=False)
    dwT = nc.declare_dram_parameter("dwT", [3, 128, 128], BF, isOutput=False)
    w0T = nc.declare_dram_parameter("w0T", [128, 64], BF, isOutput=False)
    w1T = nc.declare_dram_parameter("w1T", [64, 64], BF, isOutput=False)
    w2T = nc.declare_dram_parameter("w2T", [64, 768], BF, isOutput=False)
    fcT = nc.declare_dram_parameter("fcT", [6, 128, 256], BF, isOutput=False)
    aT = nc.declare_dram_parameter("aT", [2, 128, 12], BF, isOutput=False)
    mask8 = nc.declare_dram_parameter("mask8", [128, 8 * 128], BF, isOutput=False)
    ident = nc.declare_dram_parameter("ident", [128, 128], BF, isOutput=False)
    # f32 vector constants, packed [128, ncols]:
    # col 0: c0b (C0*dot_b), 1: b0, 2: g0, 3: bb0, 4: b1, 5: g1, 6: bb1,
    # cols 7-12: b2 blocks, 13-14: fcb halves, 15: gcol, 16: bcol, 17: abias(8)
    vcs = nc.declare_dram_parameter("vcs", [128, 18], F32, isOutput=False)
    on2 = nc.declare_dram_parameter("on2", [128, 2], F32, isOutput=False)
    bc2 = nc.declare_dram_parameter("bc2", [2, 128], F32, isOutput=False)
    onH = nc.declare_dram_parameter("onH", [128, 4], F32, isOutput=False)
    bcH = nc.declare_dram_parameter("bcH", [4, 128], F32, isOutput=False)
    outp = nc.declare_dram_parameter("out", [8, E], F32, isOutput=True)

    with tile.TileContext(nc) as tc, ExitStack() as ctx:
        cp = ctx.enter_context(tc.tile_pool(name="const", bufs=1))
        gp = ctx.enter_context(tc.tile_pool(name="gath", bufs=2))
        wk = ctx.enter_context(tc.tile_pool(name="work", bufs=2))
        wk3 = ctx.enter_context(tc.tile_pool(name="work3", bufs=3))
        psA = ctx.enter_context(tc.tile_pool(name="psA", bufs=3, space="PSUM"))
        psB = ctx.enter_context(tc.tile_pool(name="psB", bufs=2, space="PSUM"))
        psC = ctx.enter_context(tc.tile_pool(name="psC", bufs=3, space="PSUM"))

        def load_const(dram, shape, dt, nodma=False):
            t = cp.tile(shape, dt, tag=dram.name)
            if not nodma:
                nc.sync.dma_start(t[:], dram[:])
            return t

        ident_s = load_const(ident, [128, 128], BF)
        mask8_s = load_const(mask8, [128, 8, 128], BF)
        dwT_s = load_const(dwT, [128, 3, 128], BF, nodma=True)
        w0T_s = load_const(w0T, [128, 64], BF)
        w1T_s = load_const(w1T, [64, 64], BF)
        w2T_s = load_const(w2T, [64, 768], BF)
        fcT_s = load_const(fcT, [128, 6, 256], BF, nodma=True)
        aT_s = load_const(aT, [128, 2, 12], BF, nodma=True)
        vcs_s = load_const(vcs, [128, 18], F32)
        on2_s = load_const(on2, [128, 2], F32)
        bc2_s = load_const(bc2, [2, 128], F32)
        onH_s = load_const(onH, [128, 4], F32)
        bcH_s = load_const(bcH, [4, 128], F32)
        idx_s = load_const(idxw, [128, K * (NN // 16)], I16)
        evp_s = load_const(evp, [128, (E // 128) * 3], F32)

        # fix dwT / fcT / aT loads: dram dims (a, b, c) -> sbuf tile [b?..]
        # dwT dram [3, 128, 128] (l, c, d): load per l into [128, 3, 128]
        for l in range(3):
            nc.sync.dma_start(dwT_s[:, l, :], dwT[l, :, :])
        for b in range(6):
            nc.sync.dma_start(fcT_s[:, b, :], fcT[b, :, :])
        for h in range(2):
            nc.sync.dma_start(aT_s[:, h, :], aT[h, :, :])

        selfG = cp.tile([128, 8, NCH], BF, tag="selfG")
        for j in range(8):
            nc.sync.dma_start(selfG[:, j, :], selftbl[j * 128:(j + 1) * 128, :])

        c0b = vcs_s[:, 0:1]
        b0c = vcs_s[:64, 1:2]
        g0c = vcs_s[:64, 2:3]
        bb0c = vcs_s[:64, 3:4]
        b1c = vcs_s[:64, 4:5]
        g1c = vcs_s[:64, 5:6]
        bb1c = vcs_s[:64, 6:7]
        gcol = vcs_s[:, 15:16]
        bcol = vcs_s[:, 16:17]


        # ---- precompute Y0self [128d, 1024n] = C0*(W0 @ selftbl_m0^T) + C0*b ----
        s0s = cp.tile([128, 8, 128], BF, tag="s0s")
        for j in range(8):
            ps = psC.tile([128, 128], F32, tag="small")
            nc.tensor.matmul(ps[:], selfG[:, j, 0:128], ident_s[:], start=True, stop=True)
            nc.vector.tensor_copy(out=s0s[:, j, :], in_=ps[:])
        y0self = cp.tile([128, 1024], BF, tag="y0self")
        for h in range(2):
            ps = psA.tile([128, 512], F32, tag="mm")
            nc.tensor.matmul(ps[:], dwT_s[:, 0, :],
                             s0s[:].rearrange("p j c -> p (j c)")[:, h * 512:(h + 1) * 512],
                             start=True, stop=True)
            nc.scalar.add(y0self[:, h * 512:(h + 1) * 512], ps[:], add=c0b)

        M_OF_L = {1: [1, 2, 3], 2: [4, 5, 6, 7, 8]}

        for k in range(kmax):
            G = gp.tile([128, 8, NCH], BF, tag="G")
            nc.gpsimd.dma_gather(G[:], tbl[:], idx_s[:, k * 64:(k + 1) * 64],
                                 NN, NN, NCH)
            for half in range(2):
                ch = k * 2 + half          # chunk id
                col0 = ch * CHUNK          # global edge col
                tv = ch * (CHUNK // 128) * 3   # evp col offset (4 tiles * 3)

                # ---------------- sh [128, 4, 9] ----------------
                sh = wk3.tile([128, 4, 9], F32, tag="sh")
                shw = wk3.tile([128, 4, 4], F32, tag="shw")  # xx, zz, yy, n2
                evs = _bap(evp_s[:, tv:tv + 12], [evp_s.ap[0], [3, 4], [1, 3]])
                sq = wk3.tile([128, 4, 3], F32, tag="sq")
                nc.vector.tensor_mul(sq[:], evs, evs)
                n2 = shw[:, :, 3]
                nc.vector.tensor_reduce(n2, sq[:], mybir.AxisListType.X, mybir.AluOpType.add)
                nc.vector.tensor_scalar_add(n2, n2, 1e-20)
                rn2 = wk3.tile([128, 4], F32, tag="rn2")
                nc.vector.reciprocal_approx_fast(rn2[:], n2)
                nc.scalar.sqrt(rn2[:], rn2[:])          # 1/norm
                for t in range(4):
                    nc.vector.tensor_scalar_mul(sh[:, t, 1:4],
                                                evp_s[:, tv + 3 * t:tv + 3 * t + 3],
                                                rn2[:, t:t + 1])
                ux, uy, uz = sh[:, :, 1], sh[:, :, 2], sh[:, :, 3]
                nc.vector.tensor_mul(sh[:, :, 4], ux, uz)
                nc.vector.tensor_mul(sh[:, :, 5], ux, uy)
                nc.vector.tensor_mul(sh[:, :, 7], uy, uz)
                nc.vector.tensor_mul(shw[:, :, 0], ux, ux)
                nc.vector.tensor_mul(shw[:, :, 1], uz, uz)
                nc.vector.tensor_mul(shw[:, :, 2], uy, uy)
                axz = wk3.tile([128, 4], F32, tag="axz")
                nc.vector.tensor_add(axz[:], shw[:, :, 0], shw[:, :, 1])
                nc.vector.scalar_tensor_tensor(out=sh[:, :, 6], in0=axz[:], scalar=-0.5,
                                               in1=shw[:, :, 2],
                                               op0=mybir.AluOpType.mult,
                                               op1=mybir.AluOpType.add)
                nc.vector.tensor_sub(sh[:, :, 8], shw[:, :, 1], shw[:, :, 0])

                # ---------------- D matrices per tile ----------------
                Ds = []
                for t in range(4):
                    D = wk.tile([128, 8, 128], BF, tag=f"D{t}")
                    eng = nc.vector if t % 2 == 0 else nc.gpsimd
                    for mi in range(8):
                        eng.tensor_scalar_mul(D[:, mi, :], mask8_s[:, mi, :],
                                              sh[:, t, 1 + mi:2 + mi])
                    Ds.append(D)

                # ---------------- combine S blocks ----------------
                # blocks: 0=self0(precomputed) 1=neigh0 2=self1 3=neigh1 4=self2 5=neigh2
                s_sb = {}
                cpeng = [nc.scalar, nc.vector]
                bi = 0
                for blk, (src, l) in {1: ("n", 0), 2: ("s", 1), 3: ("n", 1),
                                      4: ("s", 2), 5: ("n", 2)}.items():
                    ps = psA.tile([128, 512], F32, tag="mm")
                    for t in range(4):
                        j = half * 4 + t
                        lhs_base = G if src == "n" else selfG
                        oap = ps[:, t * 128:(t + 1) * 128]
                        if l == 0:
                            nc.tensor.matmul(oap, lhs_base[:, j, 0:128], ident_s[:],
                                             start=True, stop=True)
                        else:
                            ms = M_OF_L[l]
                            for i, m in enumerate(ms):
                                nc.tensor.matmul(oap, lhs_base[:, j, m * 128:(m + 1) * 128],
                                                 Ds[t][:, m - 1, :],
                                                 start=(i == 0), stop=(i == len(ms) - 1))
                    sb = wk.tile([128, 512], BF, tag=f"ssb{blk}")
                    eng = cpeng[bi % 2]; bi += 1
                    if eng is nc.scalar:
                        nc.scalar.copy(sb[:], ps[:])
                    else:
                        eng.tensor_copy(out=sb[:], in_=ps[:])
                    s_sb[blk] = sb

                # ---------------- radial MLP ----------------
                xt = wk.tile([128, 512], BF, tag="xt")
                nc.sync.dma_start(xt[:], xT[:, col0:col0 + CHUNK])
                p0 = psC.tile([64, 512], F32, tag="small")
                nc.tensor.matmul(p0[:], w0T_s[:], xt[:], start=True, stop=True)

                def ln_block(pin, bcolv, gcolv, bbcolv):
                    stk = wk.tile([128, 512], F32, tag="stk")
                    nc.scalar.add(stk[:64, :], pin[:], add=bcolv)
                    nc.scalar.activation(out=stk[64:128, :], in_=pin[:], func=AF.Square,
                                         bias=bcolv, scale=1.0)
                    stp = psC.tile([2, 512], F32, tag="small")
                    nc.tensor.matmul(stp[:], on2_s[:], stk[:], start=True, stop=True)
                    sts = wk.tile([2, 512], F32, tag="stsrad")
                    nc.vector.tensor_copy(out=sts[:], in_=stp[:])
                    bcp = psB.tile([128, 512], F32, tag="bc")
                    nc.tensor.matmul(bcp[:], bc2_s[:], sts[:], start=True, stop=True)
                    mu = bcp[0:64, :]
                    s2 = bcp[64:128, :]
                    musq = wk.tile([64, 512], F32, tag="musq")
                    nc.scalar.square(musq[:], mu)
                    nc.vector.scalar_tensor_tensor(out=musq[:], in0=s2, scalar=EPS,
                                                   in1=musq[:],
                                                   op0=mybir.AluOpType.add,
                                                   op1=mybir.AluOpType.subtract)
                    nc.vector.reciprocal_approx_fast(musq[:], musq[:])
                    nc.scalar.sqrt(musq[:], musq[:])      # rsig [64, 512]
                    nc.vector.tensor_sub(stk[:64, :], stk[:64, :], mu)
                    t2 = wk.tile([64, 512], F32, tag="t2r")
                    nc.vector.tensor_mul(t2[:], stk[:64, :], musq[:])
                    ho = wk.tile([64, 512], BF, tag="ho")
                    if SILU_NATIVE:
                        nc.scalar.activation(out=ho[:], in_=t2[:], func=AF.Silu,
                                             bias=bbcolv, scale=gcolv)
                    else:
                        sg = wk.tile([64, 512], F32, tag="sg")
                        nc.scalar.activation(out=sg[:], in_=t2[:], func=AF.Sigmoid,
                                             bias=bbcolv, scale=gcolv)
                        ym = wk.tile([64, 512], F32, tag="ym")
                        nc.scalar.activation(out=ym[:], in_=t2[:], func=AF.Identity,
                                             bias=bbcolv, scale=gcolv)
                        nc.vector.tensor_mul(ho[:], ym[:], sg[:])
                    return ho

                h0 = ln_block(p0, b0c, g0c, bb0c)
                p1 = psC.tile([64, 512], F32, tag="small")
                nc.tensor.matmul(p1[:], w1T_s[:], h0[:], start=True, stop=True)
                h1 = ln_block(p1, b1c, g1c, bb1c)

                m0 = wk.tile([128, 6, 512], BF, tag="m0")
                for b in range(6):
                    pm = psA.tile([128, 512], F32, tag="mm")
                    nc.tensor.matmul(pm[:], w2T_s[:, b * 128:(b + 1) * 128], h1[:],
                                     start=True, stop=True)
                    b2c = vcs_s[:, 7 + b:8 + b]
                    if b % 2 == 0:
                        nc.scalar.add(m0[:, b, :], pm[:], add=b2c)
                    else:
                        nc.vector.tensor_scalar_add(m0[:, b, :], pm[:], b2c)

                # ---------------- x0 * m0 ----------------
                x0m = wk.tile([128, 6, 512], BF, tag="x0m")
                nc.vector.tensor_mul(x0m[:, 0, :], y0self[:, half * 512:(half + 1) * 512],
                                     m0[:, 0, :])
                for blk, (src, l) in {1: ("n", 0), 2: ("s", 1), 3: ("n", 1),
                                      4: ("s", 2), 5: ("n", 2)}.items():
                    yp = psA.tile([128, 512], F32, tag="mm")
                    nc.tensor.matmul(yp[:], dwT_s[:, l, :], s_sb[blk][:],
                                     start=True, stop=True)
                    if blk == 1:
                        nc.vector.scalar_tensor_tensor(out=x0m[:, 1, :], in0=yp[:],
                                                       scalar=c0b, in1=m0[:, 1, :],
                                                       op0=mybir.AluOpType.add,
                                                       op1=mybir.AluOpType.mult)
                    else:
                        nc.vector.tensor_mul(x0m[:, blk, :], yp[:], m0[:, blk, :])

                # ---------------- fc + final LN + alpha ----------------
                apsL = []
                for h2 in range(2):
                    zp = psA.tile([128, 512], F32, tag="mm")
                    for b in range(6):
                        nc.tensor.matmul(zp[:], fcT_s[:, b, h2 * 128:(h2 + 1) * 128],
                                         x0m[:, b, :], start=(b == 0), stop=(b == 5))
                    fcbc = vcs_s[:, 13 + h2:14 + h2]
                    zc = wk.tile([128, 512], F32, tag="zc")
                    nc.scalar.add(zc[:], zp[:], add=fcbc)
                    zsq = wk.tile([128, 512], F32, tag="zsq")
                    nc.vector.tensor_mul(zsq[:], zc[:], zc[:])
                    stpA = psC.tile([4, 512], F32, tag="small")
                    nc.tensor.matmul(stpA[:], onH_s[:], zc[:], start=True, stop=True)
                    stpB = psC.tile([4, 512], F32, tag="small")
                    nc.tensor.matmul(stpB[:], onH_s[:], zsq[:], start=True, stop=True)
                    stsA = wk.tile([4, 512], F32, tag="stsHa")
                    nc.vector.tensor_copy(out=stsA[:], in_=stpA[:])
                    stsB = wk.tile([4, 512], F32, tag="stsHb")
                    nc.vector.tensor_copy(out=stsB[:], in_=stpB[:])
                    musq = wk.tile([4, 512], F32, tag="musqH")
                    nc.vector.tensor_mul(musq[:], stsA[:], stsA[:])
                    nc.vector.scalar_tensor_tensor(out=musq[:], in0=stsB[:], scalar=EPS,
                                                   in1=musq[:],
                                                   op0=mybir.AluOpType.add,
                                                   op1=mybir.AluOpType.subtract)
                    nc.vector.reciprocal_approx_fast(musq[:], musq[:])
                    rsigB = wk.tile([4, 512], F32, tag="rsigH")
                    nc.scalar.sqrt(rsigB[:], musq[:])
                    mbp = psB.tile([128, 512], F32, tag="bc")
                    nc.tensor.matmul(mbp[:], bcH_s[:], stsA[:], start=True, stop=True)
                    rbp = psB.tile([128, 512], F32, tag="bc")
                    nc.tensor.matmul(rbp[:], bcH_s[:], rsigB[:], start=True, stop=True)
                    nc.vector.tensor_sub(zc[:], zc[:], mbp[:])
                    t2 = wk.tile([128, 512], BF, tag="t2H")
                    nc.vector.tensor_mul(t2[:], zc[:], rbp[:])
                    aps = psC.tile([4, 512], F32, tag="small")
                    if SILU_NATIVE:
                        sil = wk.tile([128, 512], BF, tag="silH")
                        nc.scalar.activation(out=sil[:], in_=t2[:], func=AF.Silu,
                                             bias=bcol, scale=gcol)
                        nc.tensor.matmul(aps[:], aT_s[:, h2, 0:4], t2[:], start=True, stop=False)
                        nc.tensor.matmul(aps[:], aT_s[:, h2, 4:8], sil[:], start=False, stop=True)
                    else:
                        sg = wk.tile([128, 512], BF, tag="sgH")
                        nc.scalar.activation(out=sg[:], in_=t2[:], func=AF.Sigmoid,
                                             bias=bcol, scale=gcol)
                        q = wk.tile([128, 512], BF, tag="qH")
                        nc.vector.tensor_mul(q[:], t2[:], sg[:])
                        nc.tensor.matmul(aps[:], aT_s[:, h2, 0:4], t2[:], start=True, stop=False)
                        nc.tensor.matmul(aps[:], aT_s[:, h2, 4:8], q[:], start=False, stop=False)
                        nc.tensor.matmul(aps[:], aT_s[:, h2, 8:12], sg[:], start=False, stop=True)
                    apsL.append(aps)

                for h2 in range(2):
                    asb = wk.tile([4, 512], F32, tag="asb")
                    ab = vcs_s[0:4, 17:18] if h2 == 0 else vcs_s[32:36, 17:18]
                    nc.scalar.add(asb[:], apsL[h2][:], add=ab)
                    nc.sync.dma_start(outp[h2 * 4:(h2 + 1) * 4, col0:col0 + CHUNK], asb[:])

    nc.compile()
    return nc


_NC = None


def _get_nc():
    global _NC
    if _NC is None:
        _NC = _build_nc()
    return _NC


def _host_prep(x_edge, node_irreps_input, edge_vec, f_sparse_idx_node,
               dot_w, dot_b, rad_w0, rad_b0, rad_w1, rad_b1, rad_w2, rad_b2,
               rad_g0, rad_bb0, rad_g1, rad_bb1, fc_w, fc_b, ln_g, ln_b, alpha_dot):
    f32 = np.float32
    tbl = np.ascontiguousarray(node_irreps_input.reshape(N, NCH)).astype(BF16)

    dwTn = np.zeros((3, 128, 128), f32)
    for l, s in enumerate([C0, C1, C2]):
        dwTn[l] = dot_w[l].T * s
    dwTn = dwTn.astype(BF16)

    w0Tn = rad_w0.T.astype(BF16)
    w1Tn = rad_w1.T.astype(BF16)
    w2Tn = rad_w2.T.astype(BF16)
    fcTn = np.ascontiguousarray(fc_w.T.reshape(6, 128, 256)).astype(BF16)

    aTn = np.zeros((2, 128, 12), f32)
    for hf in range(2):
        for hd in range(128):
            h_loc, dd = hd // 32, hd % 32
            a = alpha_dot[4 * hf + h_loc, dd]
            aTn[hf, hd, h_loc] = NEG * a * ln_g[dd]
            if SILU_NATIVE:
                aTn[hf, hd, 4 + h_loc] = (1 - NEG) * a
            else:
                aTn[hf, hd, 4 + h_loc] = (1 - NEG) * a * ln_g[dd]
                aTn[hf, hd, 8 + h_loc] = (1 - NEG) * a * ln_b[dd]
    aTn = aTn.astype(BF16)

    mask8n = np.zeros((128, 8 * 128), f32)
    diagv = [1.0, 1.0, 1.0, S3, S3, 1.0, S3, 0.5 * S3]  # m=1..8
    for mi in range(8):
        for p in range(128):
            mask8n[p, mi * 128 + p] = diagv[mi]
    mask8n = mask8n.astype(BF16)

    identn = np.eye(128, dtype=f32).astype(BF16)

    vcsn = np.zeros((128, 18), f32)
    vcsn[:, 0] = C0 * dot_b
    vcsn[:64, 1] = rad_b0
    vcsn[:64, 2] = rad_g0
    vcsn[:64, 3] = rad_bb0
    vcsn[:64, 4] = rad_b1
    vcsn[:64, 5] = rad_g1
    vcsn[:64, 6] = rad_bb1
    for b in range(6):
        vcsn[:, 7 + b] = rad_b2[b * 128:(b + 1) * 128]
    for h2 in range(2):
        vcsn[:, 13 + h2] = fc_b[h2 * 128:(h2 + 1) * 128]
    vcsn[:, 15] = np.tile(ln_g, 4)
    vcsn[:, 16] = np.tile(ln_b, 4)
    ab = NEG * (alpha_dot @ ln_b)
    vcsn[0:4, 17] = ab[0:4]
    vcsn[32:36, 17] = ab[4:8]

    on2n = np.zeros((128, 2), f32)
    on2n[:64, 0] = 1.0 / 64
    on2n[64:, 1] = 1.0 / 64
    bc2n = np.zeros((2, 128), f32)
    bc2n[0, :64] = 1.0
    bc2n[1, 64:] = 1.0
    onHn = np.zeros((128, 4), f32)
    for h in range(4):
        onHn[h * 32:(h + 1) * 32, h] = 1.0 / 32
    bcHn = np.zeros((4, 128), f32)
    for c in range(128):
        bcHn[c // 32, c] = 1.0
    shared = dict(tbl=tbl, dwT=dwTn, w0T=w0Tn, w1T=w1Tn, w2T=w2Tn, fcT=fcTn,
                  aT=aTn, mask8=mask8n, ident=identn, vcs=vcsn, on2=on2n,
                  bc2=bc2n, onH=onHn, bcH=bcHn)

    in_maps = []
    for c in range(NCORES):
        n0 = c * NN
        sl = slice(n0, n0 + NN)
        xc = x_edge[sl].astype(BF16)                     # [NN, K, 128]
        xTn = np.ascontiguousarray(np.transpose(xc, (2, 1, 0)).reshape(128, E))
        ev = edge_vec[sl].astype(f32)                    # [NN, K, 3]
        evkm = np.transpose(ev, (1, 0, 2)).reshape(E, 3)  # k-major [E, 3]
        evpn = np.ascontiguousarray(
            np.transpose(evkm.reshape(E // 128, 128, 3), (1, 0, 2)).reshape(128, (E // 128) * 3))
        idx = f_sparse_idx_node[sl].astype(np.int64).T.reshape(K, NN)  # k-major
        idxwn = np.zeros((128, K * (NN // 16)), np.int16)
        w = idx.reshape(K, NN // 16, 16).transpose(0, 2, 1)  # [K, 16, 64]
        for rep in range(8):
            idxwn[rep * 16:(rep + 1) * 16, :] = w.transpose(1, 0, 2).reshape(16, K * (NN // 16))
        selftbln = tbl[sl]
        m = dict(shared)
        m.update(xT=xTn, evp=evpn, idxw=idxwn, selftbl=selftbln)
        in_maps.append(m)
    return in_maps


def _assemble(results):
    full = np.zeros((N, K, NH), np.float32)
    for c in range(NCORES):
        o = results[c]["out"]                    # [8, E]
        full[c * NN:(c + 1) * NN] = np.transpose(o.reshape(NH, K, NN), (2, 1, 0))
    return full


def kernel(**inputs):
    nc = _get_nc()
    in_maps = _host_prep(**inputs)
    res = run_bass_kernel_spmd(nc, in_maps, core_ids=list(range(NCORES)))
    return _assemble(res.results)


if __name__ == "__main__":
    # quick single-core CoreSim correctness check on a reduced problem is not
    # practical (shapes hardcoded); use test.py against the reference instead.
    pass



# revision 13
# speedup vs baseline: 1.6791x; 1.6791x over previous
"""Trainium2 Bass kernel for nn_DotAlphaModule (sparse attention alpha), v2.

Strategy (8 NeuronCores, SPMD), instruction-count optimized:
  - Host pre-projects the node table: tblP[n, m*128+d] = C_l*(raw @ W_l^T)
    (+ C0*b for the l=0 block).  The gather fetches projected rows.
  - Node-block-major edge order: chunk c = (j, kg) = node block j (128
    nodes) x 4 consecutive k.  Self-term matmuls reuse one lhsT across a
    [128, 512] block-diag rhs: 8 MMs/chunk instead of 32.
  - All 64 D diag matrices of a group built in ONE DVE op via dual
    stride-0 broadcast APs (mask ⊗ sh).
  - l0 neighbor term is scale-free: DMA transpose on idle DMA engines.
  - Scalar engine pinned to the {ln, exp, square, identity, copy} act
    table: rsqrt = exp(-.5 ln v), sigmoid = 1/(1+exp(-y)) + DVE
    reciprocal_approx_fast.  Zero activation-table reloads.
  - Radial MLP widened over 2-chunk groups; output-mean fused into
    w0/w1 as an extra output column; b2 fused into w2 as a ones row.
"""
import sys
from contextlib import ExitStack

sys.path.insert(0, "/opt/trn_rl_repo")

import numpy as np
import ml_dtypes

import concourse.bass as bass
import concourse.tile as tile
import concourse.mybir as mybir
from concourse import bacc
from concourse.bass_utils import run_bass_kernel_spmd

BF16 = ml_dtypes.bfloat16

N, K = 8192, 32
NCORES = 8
NN = N // NCORES           # 1024 nodes per core
E = NN * K                 # 32768 edges per core
NCH = 9 * 128
CHUNK = 512
NCHUNK = E // CHUNK        # 64: chunk c = j*8 + kg
NG = NCHUNK // 2           # 32 groups (gather/sh/radial granularity)
EPS = 1e-5

C0 = 0.28209479177387814
C1 = 0.4886025119029199
C2 = 0.6307831305050401
S3 = 1.7320508075688772
NEG = 0.2

F32 = mybir.dt.float32
BF = mybir.dt.bfloat16
I16 = mybir.dt.int16
AF = mybir.ActivationFunctionType
ALU = mybir.AluOpType
AX = mybir.AxisListType
F32R = mybir.dt.float32r


def _brd(sl, dims):
    """AP with extra/broadcast dims appended after the partition dim."""
    return bass.AP(tensor=sl.tensor, offset=sl.offset, ap=[sl.ap[0]] + dims)


def _build_nc():
    nc = bacc.Bacc("TRN2")
    tbl = nc.declare_dram_parameter("tbl", [N, NCH], BF, isOutput=False)
    selftbl = nc.declare_dram_parameter("selftbl", [NN, NCH], BF, isOutput=False)
    y0T = nc.declare_dram_parameter("y0T", [128, NN], BF, isOutput=False)
    idxw = nc.declare_dram_parameter("idxw", [128, NG * 64], I16, isOutput=False)
    xT = nc.declare_dram_parameter("xT", [128, E], BF, isOutput=False)
    evp = nc.declare_dram_parameter("evp", [128, NG * 24], F32, isOutput=False)
    mask8 = nc.declare_dram_parameter("mask8", [128, 8 * 128], BF, isOutput=False)
    w0T = nc.declare_dram_parameter("w0T", [128, 65], BF, isOutput=False)
    w1T = nc.declare_dram_parameter("w1T", [64, 65], BF, isOutput=False)
    w2T = nc.declare_dram_parameter("w2T", [65, 768], BF, isOutput=False)
    fcT = nc.declare_dram_parameter("fcT", [6, 128, 256], BF, isOutput=False)
    aT = nc.declare_dram_parameter("aT", [128, 8], BF, isOutput=False)
    on64 = nc.declare_dram_parameter("on64", [64, 1], BF, isOutput=False)
    bc2 = nc.declare_dram_parameter("bc2", [64, 128], BF, isOutput=False)
    onAB = nc.declare_dram_parameter("onAB", [128, 80], BF, isOutput=False)
    bcF = nc.declare_dram_parameter("bcF", [128, 512], BF, isOutput=False)
    # f32 per-partition constant columns (see _host_prep for layout)
    vcs = nc.declare_dram_parameter("vcs", [128, 19], F32, isOutput=False)
    outp = nc.declare_dram_parameter("out", [8, E], F32, isOutput=True)

    with tile.TileContext(nc) as tc, ExitStack() as ctx:
        cp = ctx.enter_context(tc.tile_pool(name="const", bufs=1))
        gp = ctx.enter_context(tc.tile_pool(name="gath", bufs=2))
        wk = ctx.enter_context(tc.tile_pool(name="work", bufs=2))
        rd = ctx.enter_context(tc.tile_pool(name="rad", bufs=1))
        psS = ctx.enter_context(tc.tile_pool(name="psS", bufs=3, space="PSUM"))
        psW = ctx.enter_context(tc.tile_pool(name="psW", bufs=3, space="PSUM"))
        psF = ctx.enter_context(tc.tile_pool(name="psF", bufs=1, space="PSUM"))

        def load_const(dram, shape, dt, nodma=False):
            t = cp.tile(shape, dt, tag=dram.name, name=dram.name)
            if not nodma:
                nc.sync.dma_start(t[:], dram[:])
            return t

        mask8_s = load_const(mask8, [128, 8, 128], BF)
        y0T_s = load_const(y0T, [128, NN], BF)
        w0T_s = load_const(w0T, [128, 65], BF)
        w1T_s = load_const(w1T, [64, 65], BF)
        w2T_s = load_const(w2T, [65, 768], BF)
        fcT_s = load_const(fcT, [128, 6, 256], BF, nodma=True)
        aT_s = load_const(aT, [128, 8], BF)
        on64_s = load_const(on64, [64, 1], BF)
        bc2_s = load_const(bc2, [64, 128], BF)
        onAB_s = load_const(onAB, [128, 80], BF)
        bcF_s = load_const(bcF, [128, 512], BF)
        vcs_s = load_const(vcs, [128, 19], F32)
        evp_s = load_const(evp, [128, NG * 24], F32)
        idx_s = load_const(idxw, [128, NG * 64], I16)
        for b in range(6):
            nc.sync.dma_start(fcT_s[:, b, :], fcT[b, :, :])
        selfG = cp.tile([128, 8, NCH], BF, tag="selfG")
        for jj in range(8):
            nc.sync.dma_start(selfG[:, jj, :], selftbl[jj * 128:(jj + 1) * 128, :])

        b0c = vcs_s[:64, 0:1]
        g0c = vcs_s[:64, 1:2]
        ng0c = vcs_s[:64, 2:3]
        bb0c = vcs_s[:64, 3:4]
        nbb0c = vcs_s[:64, 4:5]
        b1c = vcs_s[:64, 5:6]
        g1c = vcs_s[:64, 6:7]
        ng1c = vcs_s[:64, 7:8]
        bb1c = vcs_s[:64, 8:9]
        nbb1c = vcs_s[:64, 9:10]
        fcb0 = vcs_s[:, 10:11]
        fcb1 = vcs_s[:, 11:12]
        g02 = vcs_s[:, 12:13]
        b02 = vcs_s[:, 13:14]
        ngF = vcs_s[:, 14:15]
        nbF = vcs_s[:, 15:16]
        b0m = vcs_s[0:1, 16:17]
        b1m = vcs_s[0:1, 17:18]
        epsR = vcs_s[0:1, 18:19]
        epsF = vcs_s[0:64, 18:19]

        def radial_layer(inw, win, bcol, bmean, gcol, ngcol, bbcol, nbbcol,
                         houtw, tagp):
            psb = rd.tile([65, 2, 512], F32, tag=f"psb{tagp}", name="psb")
            for s in range(2):
                pw = psW.tile([128, 512], F32, tag="W", name="pw")
                nc.tensor.matmul(pw[:65, :], win[:], inw[:, s, :],
                                 start=True, stop=True)
                nc.scalar.copy(psb[:, s, :], pw[:65, :])
            sqs = rd.tile([64, 2, 512], BF, tag="sqs", name="sqs")
            nc.scalar.activation(out=sqs[:].rearrange("p a b -> p (a b)"),
                                 in_=psb[:64, :, :].rearrange("p a b -> p (a b)"),
                                 func=AF.Square, bias=bcol, scale=1.0)
            rstat = rd.tile([128, 2, 512], BF, tag=f"rstat{tagp}", name="rstat")
            nc.scalar.activation(out=rstat[0:1, :, :].rearrange("p a b -> p (a b)"),
                                 in_=psb[64:65, :, :].rearrange("p a b -> p (a b)"),
                                 func=AF.Identity, bias=bmean, scale=1.0)
            m2 = rstat[64:65, :, :]
            nc.scalar.activation(out=m2[:].rearrange("p a b -> p (a b)"),
                                 in_=psb[64:65, :, :].rearrange("p a b -> p (a b)"),
                                 func=AF.Square, bias=bmean, scale=1.0)
            varw = rstat[96:97, :, :]
            for s in range(2):
                pst = psW.tile([128, 512], F32, tag="W", name="pst")
                nc.tensor.matmul(pst[:1, :], on64_s[:], sqs[:, s, :],
                                 start=True, stop=True)
                nc.vector.tensor_sub(varw[:, s, :], pst[0:1, :], m2[:, s, :])
            nc.scalar.activation(out=varw[:].rearrange("p a b -> p (a b)"),
                                 in_=varw[:].rearrange("p a b -> p (a b)"),
                                 func=AF.Ln, bias=epsR, scale=1.0)
            nc.scalar.activation(out=rstat[32:33, :, :].rearrange("p a b -> p (a b)"),
                                 in_=varw[:].rearrange("p a b -> p (a b)"),
                                 func=AF.Exp, scale=-0.5)
            xnw = rd.tile([64, 2, 512], F32, tag="xnw", name="xnw")
            for s in range(2):
                pbc = psW.tile([128, 512], F32, tag="W", name="pbc")
                nc.tensor.matmul(pbc[:], bc2_s[:], rstat[0:64, s, :],
                                 start=True, stop=True)
                nc.vector.tensor_sub(xnw[:, s, :], psb[:64, s, :], pbc[0:64, :])
                nc.vector.scalar_tensor_tensor(
                    out=xnw[:, s, :], in0=xnw[:, s, :], scalar=bcol,
                    in1=pbc[64:128, :], op0=ALU.add, op1=ALU.mult)
            yw = rd.tile([64, 2, 512], BF, tag="yw", name="yw")
            nc.scalar.activation(out=yw[:].rearrange("p a b -> p (a b)"),
                                 in_=xnw[:].rearrange("p a b -> p (a b)"),
                                 func=AF.Identity, bias=bbcol, scale=gcol)
            ew = rd.tile([64, 2, 512], F32, tag="ew", name="ew")
            nc.scalar.activation(out=ew[:].rearrange("p a b -> p (a b)"),
                                 in_=xnw[:].rearrange("p a b -> p (a b)"),
                                 func=AF.Exp, bias=nbbcol, scale=ngcol)
            nc.vector.tensor_scalar_add(ew[:].rearrange("p a b -> p (a b)"),
                                        ew[:].rearrange("p a b -> p (a b)"), 1.0)
            rw = rd.tile([64, 2, 512], F32, tag="rw", name="rw")
            nc.vector.reciprocal_approx_fast(
                rw[:].rearrange("p a b -> p (a b)"),
                ew[:].rearrange("p a b -> p (a b)"))
            nc.vector.tensor_mul(houtw[:64, :, :].rearrange("p a b -> p (a b)"),
                                 yw[:].rearrange("p a b -> p (a b)"),
                                 rw[:].rearrange("p a b -> p (a b)"))

        # one-time zero-init of sparse-layout stat tiles (bufs=1 => stable)
        fA0 = wk.tile([128, 512], BF, tag="fstatA", name="fstatA0", bufs=1)
        nc.vector.memset(fA0[:], 0.0)
        rs0a = rd.tile([128, 2, 512], BF, tag="rstata", name="rstata0")
        nc.vector.memset(rs0a[:].rearrange("p a b -> p (a b)"), 0.0)
        rs0b = rd.tile([128, 2, 512], BF, tag="rstatb", name="rstatb0")
        nc.vector.memset(rs0b[:].rearrange("p a b -> p (a b)"), 0.0)

        for g in range(NG):
            j = g // 4
            G2 = gp.tile([128, 8, NCH], BF, tag="G2", name="G2")
            nc.gpsimd.dma_gather(G2[:], tbl[:], idx_s[:, g * 64:(g + 1) * 64],
                                 NN, NN, NCH)

            # ---- sh: [128, 8kk, 8m] ----
            evsl = evp_s[:, g * 24:(g + 1) * 24]
            ev = _brd(evsl, [[3, 8], [1, 3]])
            sq = wk.tile([128, 8, 3], F32, tag="sq", name="sq")
            nc.vector.tensor_mul(sq[:], ev, ev)
            n2 = wk.tile([128, 8], F32, tag="n2", name="n2")
            nc.vector.tensor_reduce(n2[:], sq[:], AX.X, ALU.add)
            rn = wk.tile([128, 8], F32, tag="rn", name="rn")
            nc.scalar.activation(out=rn[:], in_=n2[:], func=AF.Ln, scale=1.0)
            nc.scalar.activation(out=rn[:], in_=rn[:], func=AF.Exp, scale=-0.5)
            rn2 = wk.tile([128, 8], F32, tag="rn2", name="rn2")
            nc.vector.reciprocal_approx_fast(rn2[:], n2[:])
            sh = wk.tile([128, 8, 8], F32, tag="sh", name="sh")
            nc.vector.tensor_mul(sh[:, :, 0:3], ev, _brd(rn, [[1, 8], [0, 3]]))
            nc.vector.tensor_mul(sh[:, :, 3:5],
                                 _brd(evsl, [[3, 8], [0, 2]]),
                                 _brd(evsl[:, 2:], [[3, 8], [-1, 2]]))
            nc.vector.tensor_mul(sh[:, :, 6:7],
                                 _brd(evsl[:, 1:], [[3, 8], [1, 1]]),
                                 _brd(evsl[:, 2:], [[3, 8], [1, 1]]))
            axz = wk.tile([128, 8, 1], F32, tag="axz", name="axz")
            nc.vector.tensor_add(axz[:], sq[:, :, 0:1], sq[:, :, 2:3])
            nc.vector.scalar_tensor_tensor(
                out=sh[:, :, 5:6], in0=axz[:], scalar=-0.5, in1=sq[:, :, 1:2],
                op0=ALU.mult, op1=ALU.add)
            nc.vector.tensor_sub(sh[:, :, 7:8], sq[:, :, 2:3], sq[:, :, 0:1])
            nc.vector.tensor_mul(sh[:, :, 3:8], sh[:, :, 3:8],
                                 _brd(rn2, [[1, 8], [0, 5]]))

            # ---- radial MLP for this group's 1024 edges ----
            col0 = g * 2 * CHUNK
            xt = rd.tile([128, 2, 512], BF, tag="xt", name="xt")
            nc.sync.dma_start(xt[:].rearrange("p a b -> p (a b)"),
                              xT[:, col0:col0 + 1024])
            h0w = rd.tile([64, 2, 512], BF, tag="h0w", name="h0w")
            radial_layer(xt, w0T_s, b0c, b0m, g0c, ng0c, bb0c, nbb0c, h0w, "a")
            h1w = rd.tile([65, 2, 512], BF, tag="h1w", name="h1w")
            nc.vector.memset(h1w[64:65, :, :], 1.0)
            radial_layer(h0w, w1T_s, b1c, b1m, g1c, ng1c, bb1c, nbb1c, h1w, "b")

            # ---- D: [128, 8m, 8kk, 128] in one DVE op ----
            D = wk.tile([128, 8, 8, 128], BF, tag="D", name="D", bufs=1)
            nc.vector.tensor_tensor(
                out=D[:],
                in0=_brd(mask8_s[:], [[128, 8], [0, 8], [1, 128]]),
                in1=_brd(sh[:], [[1, 8], [8, 8], [0, 128]]),
                op=ALU.mult)

            for half in range(2):
                c = g * 2 + half
                ccol = c * CHUNK
                x0 = wk.tile([128, 6, 512], BF, tag="x0", name="x0")

                # ---- combine: self then neighbor, copies interleaved ----
                Ss = psS.tile([128, 512], F32, tag="S", name="Ss")
                for i, m in enumerate((1, 2, 3)):
                    nc.tensor.matmul(
                        Ss[:], selfG[:, j, m * 128:(m + 1) * 128],
                        D[:, m - 1, half * 4:(half + 1) * 4, :].rearrange(
                            "p a b -> p (a b)"),
                        start=(i == 0), stop=(i == 2))
                nc.scalar.copy(x0[:, 2, :], Ss[:])
                Ss2 = psS.tile([128, 512], F32, tag="S", name="Ss2")
                for i, m in enumerate((4, 5, 6, 7, 8)):
                    nc.tensor.matmul(
                        Ss2[:], selfG[:, j, m * 128:(m + 1) * 128],
                        D[:, m - 1, half * 4:(half + 1) * 4, :].rearrange(
                            "p a b -> p (a b)"),
                        start=(i == 0), stop=(i == 4))
                nc.scalar.copy(x0[:, 4, :], Ss2[:])
                Sn1 = psS.tile([128, 512], F32, tag="S", name="Sn1")
                for t in range(4):
                    kk = half * 4 + t
                    for i, m in enumerate((1, 2, 3)):
                        nc.tensor.matmul(
                            Sn1[:, t * 128:(t + 1) * 128],
                            G2[:, kk, m * 128:(m + 1) * 128],
                            D[:, m - 1, kk, :],
                            start=(i == 0), stop=(i == 2))
                nc.vector.tensor_copy(out=x0[:, 3, :], in_=Sn1[:])
                Sn2 = psS.tile([128, 512], F32, tag="S", name="Sn2")
                for t in range(4):
                    kk = half * 4 + t
                    for i, m in enumerate((4, 5, 6, 7, 8)):
                        nc.tensor.matmul(
                            Sn2[:, t * 128:(t + 1) * 128],
                            G2[:, kk, m * 128:(m + 1) * 128],
                            D[:, m - 1, kk, :],
                            start=(i == 0), stop=(i == 4))
                nc.vector.tensor_copy(out=x0[:, 5, :], in_=Sn2[:])
                for t in range(4):
                    nc.sync.dma_start_transpose(
                        x0[:, 1, t * 128:(t + 1) * 128],
                        G2[:, half * 4 + t, 0:128])

                # ---- m0 = w2 @ h1 (+b2 via ones row); x0m = x0 * m0 ----
                x0m = wk.tile([128, 6, 512], BF, tag="x0m", name="x0m")
                y0sl = y0T_s[:, j * 128:(j + 1) * 128]
                for b in range(6):
                    pm = psW.tile([128, 512], F32, tag="W", name="pm")
                    nc.tensor.matmul(pm[:], w2T_s[:, b * 128:(b + 1) * 128],
                                     h1w[:, half, :], start=True, stop=True)
                    if b == 0:
                        nc.vector.tensor_mul(
                            _brd(x0m[:, 0, :], [[128, 4], [1, 128]]),
                            _brd(y0sl, [[0, 4], [1, 128]]),
                            _brd(pm[:], [[128, 4], [1, 128]]))
                    else:
                        nc.vector.tensor_mul(x0m[:, b, :], x0[:, b, :], pm[:])

                # ---- fc ----
                zp = psF.tile([128, 2, 512], F32, tag="zp", name="zp")
                for h2 in range(2):
                    for b in range(6):
                        nc.tensor.matmul(
                            zp[:, h2, :], fcT_s[:, b, h2 * 128:(h2 + 1) * 128],
                            x0m[:, b, :], start=(b == 0), stop=(b == 5))

                # ---- final LN stats ----
                zc = wk.tile([128, 2, 512], BF, tag="zc", name="zc", bufs=1)
                zq = wk.tile([128, 2, 512], BF, tag="zq", name="zq", bufs=1)
                # fstatA: mu@0:4/32:36, msq@64:68/96:100  (h0/h1)
                # fstatB: m2@0:64 (sparse), var@64+{0:4,32:36}
                # fstatC: rsig@{0:4,32:36}, murs@64+{0:4,32:36}
                fstatA = wk.tile([128, 512], BF, tag="fstatA", name="fstatA", bufs=1)
                fstatB = wk.tile([128, 512], BF, tag="fstatB", name="fstatB", bufs=1)
                fstatC = wk.tile([128, 512], BF, tag="fstatC", name="fstatC", bufs=1)
                for h2 in range(2):
                    fcb = fcb0 if h2 == 0 else fcb1
                    nc.scalar.activation(out=zc[:, h2, :], in_=zp[:, h2, :],
                                         func=AF.Identity, bias=fcb, scale=1.0)
                    nc.scalar.activation(out=zq[:, h2, :], in_=zp[:, h2, :],
                                         func=AF.Square, bias=fcb, scale=1.0)
                    stp = psW.tile([128, 512], F32, tag="W", name="stp")
                    nc.tensor.matmul(stp[:40, :], onAB_s[:, 0:40], zc[:, h2, :],
                                     start=True, stop=False)
                    nc.tensor.matmul(stp[:40, :], onAB_s[:, 40:80], zq[:, h2, :],
                                     start=False, stop=True)
                    if h2 == 0:
                        nc.scalar.copy(fstatA[0:4, :], stp[0:4, :])
                        nc.scalar.copy(fstatA[64:68, :], stp[32:36, :])
                    else:
                        nc.vector.tensor_copy(out=fstatA[32:36, :], in_=stp[0:4, :])
                        nc.vector.tensor_copy(out=fstatA[96:100, :], in_=stp[32:36, :])
                nc.scalar.square(fstatB[64:128, :], fstatA[0:64, :])
                nc.vector.tensor_sub(fstatB[0:64, :], fstatA[64:128, :],
                                     fstatB[64:128, :])
                nc.scalar.activation(out=fstatB[0:64, :], in_=fstatB[0:64, :],
                                     func=AF.Ln, bias=epsF, scale=1.0)
                nc.scalar.activation(out=fstatC[0:64, :], in_=fstatB[0:64, :],
                                     func=AF.Exp, scale=-0.5)
                nc.vector.tensor_mul(fstatC[64:128, :], fstatA[0:64, :],
                                     fstatC[0:64, :])

                t2 = wk.tile([128, 2, 512], F32, tag="t2", name="t2", bufs=1)
                for h2 in range(2):
                    pmu = psW.tile([128, 512], F32, tag="W", name="pmu")
                    nc.tensor.matmul(pmu[:], bcF_s[:, h2 * 128:(h2 + 1) * 128],
                                     fstatC[:], start=True, stop=True)
                    prs = psW.tile([128, 512], F32, tag="W", name="prs")
                    nc.tensor.matmul(prs[:],
                                     bcF_s[:, 256 + h2 * 128:256 + (h2 + 1) * 128],
                                     fstatC[:], start=True, stop=True)
                    nc.vector.tensor_mul(t2[:, h2, :], zc[:, h2, :], prs[:])
                    nc.vector.tensor_sub(t2[:, h2, :], t2[:, h2, :], pmu[:])

                # ---- smooth leaky relu + alpha ----
                y02 = wk.tile([128, 2, 512], BF, tag="y02", name="y02")
                nc.scalar.activation(out=y02[:].rearrange("p a b -> p (a b)"),
                                     in_=t2[:].rearrange("p a b -> p (a b)"),
                                     func=AF.Identity, bias=b02, scale=g02)
                e2 = wk.tile([128, 2, 512], F32, tag="e2", name="e2", bufs=1)
                nc.scalar.activation(out=e2[:].rearrange("p a b -> p (a b)"),
                                     in_=t2[:].rearrange("p a b -> p (a b)"),
                                     func=AF.Exp, bias=nbF, scale=ngF)
                nc.vector.tensor_scalar_add(e2[:].rearrange("p a b -> p (a b)"),
                                            e2[:].rearrange("p a b -> p (a b)"),
                                            1.0)
                r2 = wk.tile([128, 2, 512], F32, tag="r2", name="r2", bufs=1)
                nc.vector.reciprocal_approx_fast(
                    r2[:].rearrange("p a b -> p (a b)"),
                    e2[:].rearrange("p a b -> p (a b)"))
                q2 = wk.tile([128, 2, 512], F32, tag="q2", name="q2", bufs=1)
                nc.vector.tensor_mul(q2[:].rearrange("p a b -> p (a b)"),
                                     y02[:].rearrange("p a b -> p (a b)"),
                                     r2[:].rearrange("p a b -> p (a b)"))
                s2 = wk.tile([128, 2, 512], BF, tag="s2", name="s2")
                nc.vector.scalar_tensor_tensor(
                    out=s2[:].rearrange("p a b -> p (a b)"),
                    in0=q2[:].rearrange("p a b -> p (a b)"), scalar=4.0,
                    in1=y02[:].rearrange("p a b -> p (a b)"),
                    op0=ALU.mult, op1=ALU.add)
                asb = wk.tile([4, 2, 512], F32, tag="asb", name="asb", bufs=2)
                for h2 in range(2):
                    pal = psW.tile([128, 512], F32, tag="W", name="pal")
                    nc.tensor.matmul(pal[:4, :], aT_s[:, h2 * 4:(h2 + 1) * 4],
                                     s2[:, h2, :], start=True, stop=True)
                    if h2 == 0:
                        nc.scalar.copy(asb[:, 0, :], pal[:4, :])
                    else:
                        nc.vector.tensor_copy(out=asb[:, 1, :], in_=pal[:4, :])
                    nc.sync.dma_start(
                        outp[h2 * 4:(h2 + 1) * 4, ccol:ccol + CHUNK],
                        asb[:, h2, :])

    nc.compile()
    return nc


_NC = None


def _get_nc():
    global _NC
    if _NC is None:
        _NC = _build_nc()
    return _NC


def _host_prep(x_edge, node_irreps_input, edge_vec, f_sparse_idx_node,
               dot_w, dot_b, rad_w0, rad_b0, rad_w1, rad_b1, rad_w2, rad_b2,
               rad_g0, rad_bb0, rad_g1, rad_bb1, fc_w, fc_b, ln_g, ln_b,
               alpha_dot):
    f32 = np.float32
    x_edge = np.asarray(x_edge, f32)
    node_irreps_input = np.asarray(node_irreps_input, f32)
    edge_vec = np.asarray(edge_vec, f32)
    idx_all = np.asarray(f_sparse_idx_node)

    # projected node table: [N, 9, 128] with C_l and l0-bias folded
    CL = [C0, C1, C1, C1, C2, C2, C2, C2, C2]
    LOF = [0, 1, 1, 1, 2, 2, 2, 2, 2]
    tblP = np.empty((N, 9, 128), f32)
    for m in range(9):
        tblP[:, m, :] = CL[m] * (node_irreps_input[:, m, :] @
                                 np.asarray(dot_w, f32)[LOF[m]].T)
    tblP[:, 0, :] += C0 * np.asarray(dot_b, f32)
    tblPb = np.ascontiguousarray(tblP.reshape(N, NCH)).astype(BF16)

    mask8n = np.zeros((128, 8 * 128), f32)
    diagv = [1.0, 1.0, 1.0, S3, S3, 1.0, S3, 0.5 * S3]
    for mi in range(8):
        for p in range(128):
            mask8n[p, mi * 128 + p] = diagv[mi]
    mask8n = mask8n.astype(BF16)

    w0 = np.asarray(rad_w0, f32)
    w1 = np.asarray(rad_w1, f32)
    w2 = np.asarray(rad_w2, f32)
    w0Tn = np.zeros((128, 65), f32)
    w0Tn[:, :64] = w0.T
    w0Tn[:, 64] = w0.mean(axis=0)
    w1Tn = np.zeros((64, 65), f32)
    w1Tn[:, :64] = w1.T
    w1Tn[:, 64] = w1.mean(axis=0)
    w2Tn = np.zeros((65, 768), f32)
    w2Tn[:64, :] = w2.T
    w2Tn[64, :] = np.asarray(rad_b2, f32)

    fcTn = np.ascontiguousarray(np.asarray(fc_w, f32).T.reshape(6, 128, 256)
                                ).astype(BF16)

    aTn = np.zeros((128, 8), f32)
    for h2 in range(2):
        for p in range(128):
            aTn[p, h2 * 4 + p // 32] = np.asarray(alpha_dot, f32)[
                4 * h2 + p // 32, p % 32]

    on64n = np.full((64, 1), 1.0 / 64, f32)
    bc2n = np.zeros((64, 128), f32)
    bc2n[0, :64] = 1.0
    bc2n[32, 64:] = 1.0
    onABn = np.zeros((128, 80), f32)
    for p in range(128):
        onABn[p, p // 32] = 1.0 / 32          # matmul A: mu rows 0:4
        onABn[p, 40 + 32 + p // 32] = 1.0 / 32  # matmul B: msq rows 32:36
    bcFn = np.zeros((128, 512), f32)
    for h2 in range(2):
        for p in range(128):
            bcFn[64 + 32 * h2 + p // 32, h2 * 128 + p] = 1.0       # mu*rsig
            bcFn[32 * h2 + p // 32, 256 + h2 * 128 + p] = 1.0      # rsig

    gt = np.tile(np.asarray(ln_g, f32), 4)
    bt = np.tile(np.asarray(ln_b, f32), 4)
    vcsn = np.zeros((128, 19), f32)
    vcsn[:64, 0] = rad_b0
    vcsn[:64, 1] = rad_g0
    vcsn[:64, 2] = -np.asarray(rad_g0, f32)
    vcsn[:64, 3] = rad_bb0
    vcsn[:64, 4] = -np.asarray(rad_bb0, f32)
    vcsn[:64, 5] = rad_b1
    vcsn[:64, 6] = rad_g1
    vcsn[:64, 7] = -np.asarray(rad_g1, f32)
    vcsn[:64, 8] = rad_bb1
    vcsn[:64, 9] = -np.asarray(rad_bb1, f32)
    vcsn[:, 10] = np.asarray(fc_b, f32)[0:128]
    vcsn[:, 11] = np.asarray(fc_b, f32)[128:256]
    vcsn[:, 12] = 0.2 * gt
    vcsn[:, 13] = 0.2 * bt
    vcsn[:, 14] = -gt
    vcsn[:, 15] = -bt
    vcsn[0, 16] = np.asarray(rad_b0, f32).mean()
    vcsn[0, 17] = np.asarray(rad_b1, f32).mean()
    vcsn[:, 18] = EPS

    shared = dict(tbl=tblPb, mask8=mask8n, w0T=w0Tn.astype(BF16),
                  w1T=w1Tn.astype(BF16), w2T=w2Tn.astype(BF16), fcT=fcTn,
                  aT=aTn.astype(BF16), on64=on64n.astype(BF16),
                  bc2=bc2n.astype(BF16), onAB=onABn.astype(BF16),
                  bcF=bcFn.astype(BF16), vcs=vcsn)

    in_maps = []
    for core in range(NCORES):
        n0 = core * NN
        sl = slice(n0, n0 + NN)
        selftbln = tblPb[sl]
        y0Tn = np.ascontiguousarray(tblP[sl, 0, :].T).astype(BF16)

        # xT: [128f, E] with e = j*4096 + k*128 + p
        xc = x_edge[sl]                                   # [1024, 32, 128]
        xTn = np.ascontiguousarray(
            xc.reshape(8, 128, 8, 4, 128).transpose(4, 0, 2, 3, 1)
            .reshape(128, E)).astype(BF16)

        # evp: [128p, g*24 + kk*3 + d]
        ev = edge_vec[sl].astype(f32)                     # [1024, 32, 3]
        evpn = np.ascontiguousarray(
            ev.reshape(8, 128, 4, 8, 3).transpose(1, 0, 2, 3, 4)
            .reshape(128, NG * 24))

        # idxw: gather g covers rows i = kk*128 + p ->
        #   f_idx[n0 + (g//4)*128 + p, (g%4)*8 + kk], wrapped [16, 64], x8
        idx = idx_all[sl].astype(np.int64)                # [1024, 32]
        idxg = idx.reshape(8, 128, 4, 8).transpose(0, 2, 3, 1)  # [j, g4, kk, p]
        flat = idxg.reshape(32, 1024)                     # [g, i] i = kk*128+p
        wrap = flat.reshape(32, 64, 16).transpose(0, 2, 1)  # [g, s, col]
        idxwn = np.zeros((128, NG * 64), np.int16)
        blk = wrap.transpose(1, 0, 2).reshape(16, NG * 64)
        for rep in range(8):
            idxwn[rep * 16:(rep + 1) * 16, :] = blk
        m = dict(shared)
        m.update(selftbl=selftbln, y0T=y0Tn, xT=xTn, evp=evpn, idxw=idxwn)
        in_maps.append(m)
    return in_maps


def _assemble(results):
    full = np.zeros((N, K, 8), np.float32)
    for core in range(NCORES):
        o = results[core]["out"]                      # [8, E]
        full[core * NN:(core + 1) * NN] = (
            o.reshape(8, 8, 8, 4, 128).transpose(1, 4, 2, 3, 0)
            .reshape(NN, K, 8))
    return full


def kernel(**inputs):
    nc = _get_nc()
    in_maps = _host_prep(**inputs)
    res = run_bass_kernel_spmd(nc, in_maps, core_ids=list(range(NCORES)))
    return _assemble(res.results)
